# revision 2
# baseline (speedup 1.0000x reference)
"""2-layer GraphSAGE (mean aggr) on 8 Trainium2 NeuronCores.

Strategy: partition destination nodes across cores (graph parallel).
Segment-mean is computed as TensorE matmuls: for each tile of 128 gathered
source rows M [128e, D], a routing matrix S [128e, W] (one-hot by local
destination, scaled by 1/deg) accumulates aggT[k, d] += M.T @ S into PSUM
per W-node destination block. Source rows are fetched with dma_gather
(int16 indices -> 4 source chunks of 25000 rows). Linear layers and bias
are applied per block on TensorE. Layer 1 and layer 2 run as two launches;
the host assembles the full hidden table in between (the inter-core
exchange).

Node->slot assignment: nodes are degree-sorted into bands of
n_cores*W; within each band a greedy 4-vector (per-source-chunk
in-degree) balancer splits nodes across the 8 cores so each
(block, chunk) cell has near-equal edge counts on every core. This
minimizes the shared gather-tile padding (T = max over cores).

MODE "f16": W=256 destination windows; features, routing matrices and
weights in fp16 (PE: 1 cycle/row; PSUM accumulates fp32; ~5e-4 rel err).
Block outputs are produced transposed [out_d, W]; layer-1 output is
written fp16 and reused directly as the layer-2 gather table.
MODE "f32r": fp32r datapath (~1e-4). MODE "f32": everything fp32.
"""

import contextlib
import sys

sys.path.insert(0, "/opt/trn_rl_repo")

import numpy as np

import concourse.mybir as mybir
import concourse.tile as tile
from concourse import bacc, bass_utils

N_NODES = 100000
N_EDGES = 1600000
IN_DIM = 128
HID_DIM = 128
OUT_DIM = 64
N_CORES = 8
N_CHUNKS = 4
CHUNK_SZ = 25000
GATHER_MAX = 1024  # HW limit: dma_gather wedges above this

MODE = "f16"       # "f32" | "f32r" | "f16"

_plan_cache: dict = {}
_prog_cache: dict = {}


def _block_w(mode):
    return 128 if mode == "f32" else 256


def _transposed_out(mode):
    return mode in ("f32r", "f16")


def _assign_slots(deg4, n_nodes, n_cores, W):
    """Greedy per-band balance of per-chunk degree vectors across cores.

    Returns (slot_of_node, n_bands). Band j holds the degree-ranked nodes
    [j*n_cores*W, ...); within the band each core gets W nodes chosen to
    equalize the per-chunk edge counts (which become the gather cells).
    """
    tot = deg4.sum(1)
    order = np.argsort(-tot, kind="stable")
    band_sz = n_cores * W
    n_bands = -(-n_nodes // band_sz)
    spc = n_bands * W
    slot_of_node = np.empty(n_nodes, np.int64)
    BIG = np.int64(1) << 60
    for j in range(n_bands):
        nodes = order[j * band_sz: (j + 1) * band_sz]
        s = np.zeros((n_cores, deg4.shape[1]), np.int64)
        cap = np.zeros(n_cores, np.int64)
        for n_ in nodes:
            v = deg4[n_]
            cand = s + v
            post = np.maximum(cand, s.max(0))
            score = post.sum(1)
            score[cap >= W] = BIG
            k = int(score.argmin())
            slot_of_node[n_] = k * spc + j * W + cap[k]
            s[k] += v
            cap[k] += 1
    return slot_of_node, n_bands


def _make_plan(edge_index, n_nodes, n_cores, chunk_sz, n_chunks, mode=MODE):
    src = np.asarray(edge_index[0], dtype=np.int64)
    dst = np.asarray(edge_index[1], dtype=np.int64)
    n_edges = src.shape[0]
    W = _block_w(mode)

    deg = np.bincount(dst, minlength=n_nodes).astype(np.int64)
    cnt_inv = (1.0 / np.maximum(deg, 1)).astype(np.float32)

    chunk_e = src // chunk_sz
    deg4 = np.bincount(dst * n_chunks + chunk_e,
                       minlength=n_nodes * n_chunks
                       ).reshape(n_nodes, n_chunks)
    slot_of_node, bpc = _assign_slots(deg4, n_nodes, n_cores, W)
    slots_per_core = bpc * W

    dslot = slot_of_node[dst]
    core_e = dslot // slots_per_core
    blk_e = (dslot % slots_per_core) // W
    dloc_e = dslot % W

    cell = (core_e * bpc + blk_e) * n_chunks + chunk_e
    n_cells = n_cores * bpc * n_chunks
    counts = np.bincount(cell, minlength=n_cells).reshape(
        n_cores, bpc, n_chunks)
    T = -(-counts.max(axis=0) // 128)            # [bpc, n_chunks] tiles/cell

    cell_slots = (T * 128).astype(np.int64)
    seg_len = cell_slots.sum(axis=0)             # per chunk
    seg_start = np.concatenate([[0], np.cumsum(seg_len)[:-1]])
    cell_base = np.empty((bpc, n_chunks), np.int64)
    for c in range(n_chunks):
        cell_base[:, c] = seg_start[c] + np.concatenate(
            [[0], np.cumsum(cell_slots[:, c])[:-1]])
    total_slots = int(cell_slots.sum())

    gathers = []
    for c in range(n_chunks):
        lst = []
        off = 0
        while off < seg_len[c]:
            n = int(min(GATHER_MAX, seg_len[c] - off))
            lst.append((int(seg_start[c] + off), n))
            off += n
        gathers.append(lst)

    # slot position of every edge
    eorder = np.argsort(cell, kind="stable")
    sorted_cell = cell[eorder]
    group_start = np.zeros(n_edges, np.int64)
    new_grp = np.empty(n_edges, bool)
    new_grp[0] = True
    new_grp[1:] = sorted_cell[1:] != sorted_cell[:-1]
    grp_first = np.where(new_grp)[0]
    group_start[grp_first] = grp_first
    group_start = np.maximum.accumulate(group_start)
    rank = np.arange(n_edges) - group_start

    b_of = (sorted_cell // n_chunks) % bpc
    c_of = sorted_cell % n_chunks
    core_of = sorted_cell // (bpc * n_chunks)
    pos = cell_base[b_of, c_of] + rank

    idx_vals = np.zeros((n_cores, total_slots), np.int16)
    dloc_vals = np.full((n_cores, total_slots), -1.0, np.float32)
    cinv_vals = np.zeros((n_cores, total_slots), np.float32)

    es, ed = src[eorder], dst[eorder]
    idx_vals[core_of, pos] = (es - c_of * chunk_sz).astype(np.int16)
    dloc_vals[core_of, pos] = dloc_e[eorder].astype(np.float32)
    cinv_vals[core_of, pos] = cnt_inv[ed]

    idx16 = np.ascontiguousarray(
        np.tile(idx_vals.reshape(n_cores, -1, 16).transpose(0, 2, 1),
                (1, 8, 1)))
    dstloc = np.ascontiguousarray(
        dloc_vals.reshape(n_cores, -1, 128).transpose(0, 2, 1))
    cntinv = np.ascontiguousarray(
        cinv_vals.reshape(n_cores, -1, 128).transpose(0, 2, 1))

    return dict(
        slot_of_node=slot_of_node, bpc=bpc, slots_per_core=slots_per_core,
        T=T, gathers=gathers, total_slots=total_slots,
        cell_base=cell_base, seg_start=seg_start, mode=mode, W=W,
        idx16=idx16, dstloc=dstloc, cntinv=cntinv, chunk_sz=chunk_sz,
        n_chunks=n_chunks, n_nodes=n_nodes, n_cores=n_cores,
    )


def _feat_np_dtype(mode):
    return np.float16 if mode == "f16" else np.float32


def _build_program(plan, table_rows, out_d, relu, loop_k=1, out_f32=None):
    """One layer's SPMD program (shared by all cores).

    loop_k > 1 wraps the block loop in a hardware For loop repeating the
    computation loop_k times (timing only). out_f32 forces the DRAM
    output dtype (default: f32 unless mode f16 and relu, i.e. layer 1,
    whose output feeds the next layer's f16 gather table).
    """
    bpc = plan["bpc"]
    T = plan["T"]
    n_chunks = plan["n_chunks"]
    chunk_sz = plan["chunk_sz"]
    total_slots = plan["total_slots"]
    slots_pc = plan["slots_per_core"]
    gathers = plan["gathers"]
    cell_base = plan["cell_base"]
    seg_start = plan["seg_start"]
    mode = plan["mode"]
    W = plan["W"]
    D = 128
    f32 = mybir.dt.float32
    if mode == "f32r":
        mdt = mybir.dt.float32r
    elif mode == "f16":
        mdt = mybir.dt.float16
    else:
        mdt = f32
    if out_f32 is None:
        out_f32 = not (mode == "f16" and relu)
    out_dt = f32 if out_f32 else mdt
    # self-term inputs: f16 mode runs them in f16 (1 cyc/row); f32r keeps f32
    sdt = mdt if mode == "f16" else f32

    nc = bacc.Bacc("TRN2", target_bir_lowering=False, debug=False)
    with tile.TileContext(nc) as tc:
        with tc.tile_pool(name="dram", bufs=1, space="DRAM") as dram:
            table = dram.tile([table_rows, D], mdt,
                              kind="ExternalInput", name="table")
            idx16 = dram.tile([128, total_slots // 16], mybir.dt.int16,
                              kind="ExternalInput", name="idx16")
            dstloc = dram.tile([128, total_slots // 128], f32,
                               kind="ExternalInput", name="dstloc")
            cntinv = dram.tile([128, total_slots // 128], f32,
                               kind="ExternalInput", name="cntinv")
            xT = dram.tile([D, slots_pc], sdt,
                           kind="ExternalInput", name="xT")
            wl = dram.tile([D, out_d], mdt,
                           kind="ExternalInput", name="wl")
            wr = dram.tile([D, out_d], sdt,
                           kind="ExternalInput", name="wr")
            brow = dram.tile([1, out_d], mdt,
                             kind="ExternalInput", name="brow")
            iota_in = dram.tile([128, W], mdt,
                                kind="ExternalInput", name="iota")
            onesr = dram.tile([1, W], mdt,
                              kind="ExternalInput", name="onesr")
            if _transposed_out(mode):
                out = dram.tile([out_d, slots_pc], out_dt,
                                kind="ExternalOutput", name="out")
            else:
                out = dram.tile([slots_pc, out_d], out_dt,
                                kind="ExternalOutput", name="out")

        with tc.tile_pool(name="const", bufs=1) as cpool, \
             tc.tile_pool(name="gbuf", bufs=4) as gpool, \
             tc.tile_pool(name="spool", bufs=4) as spool, \
             tc.tile_pool(name="fpool", bufs=3) as fpool, \
             tc.tile_pool(name="psA", bufs=2, space="PSUM") as psA, \
             tc.tile_pool(name="psB", bufs=2, space="PSUM") as psB:

            idx_sb = cpool.tile([128, total_slots // 16], mybir.dt.int16)
            dst_sb = cpool.tile([128, total_slots // 128], f32)
            cnt_sb = cpool.tile([128, total_slots // 128], f32)
            xT_sb = cpool.tile([D, slots_pc], sdt)
            wl_sb = cpool.tile([D, out_d], mdt)
            wr_sb = cpool.tile([D, out_d], sdt)
            b_sb = cpool.tile([1, out_d], mdt)
            ones_sb = cpool.tile([1, W], mdt)
            iota_sb = cpool.tile([128, W], mdt)

            nc.sync.dma_start(out=idx_sb[:], in_=idx16[:])
            nc.sync.dma_start(out=dst_sb[:], in_=dstloc[:])
            nc.sync.dma_start(out=cnt_sb[:], in_=cntinv[:])
            nc.sync.dma_start(out=xT_sb[:], in_=xT[:])
            nc.sync.dma_start(out=wl_sb[:], in_=wl[:])
            nc.sync.dma_start(out=wr_sb[:], in_=wr[:])
            nc.sync.dma_start(out=b_sb[:], in_=brow[:])
            nc.sync.dma_start(out=iota_sb[:], in_=iota_in[:])
            nc.sync.dma_start(out=ones_sb[:], in_=onesr[:])

            loop_ctx = (tc.For_i(0, loop_k, 1) if loop_k > 1
                        else contextlib.nullcontext())

            gtiles = [dict() for _ in range(n_chunks)]
            next_g = [0] * n_chunks

            def ensure_gather(c, gi):
                while next_g[c] <= gi:
                    g = next_g[c]
                    s0, n = gathers[c][g]
                    gb = gpool.tile([128, GATHER_MAX // 128, D], mdt,
                                    tag=f"g{c}", name=f"gb_{c}_{g}")
                    nc.gpsimd.dma_gather(
                        out_ap=gb[:, : -(-n // 128), :],
                        in_ap=table[c * chunk_sz : min((c + 1) * chunk_sz,
                                                       table_rows), :],
                        idxs_ap=idx_sb[:, s0 // 16 : (s0 + n) // 16],
                        num_idxs=n,
                        num_idxs_reg=n,
                        elem_size=D,
                    )
                    gtiles[c][g] = gb
                    next_g[c] = g + 1

            stack = contextlib.ExitStack()
            stack.enter_context(loop_ctx)
            for b in range(bpc):
                agg = psA.tile([D, W], f32, space="PSUM",
                               tag="agg", name=f"agg_{b}")
                n_mm = int(T[b].sum())
                mm = 0
                for c in range(n_chunks):
                    tcount = int(T[b, c])
                    for t in range(tcount):
                        slot0 = int(cell_base[b, c]) + t * 128
                        g = (slot0 - int(seg_start[c])) // GATHER_MAX
                        tin = ((slot0 - int(seg_start[c])) % GATHER_MAX) // 128
                        ensure_gather(c, g)
                        gb = gtiles[c][g]
                        gt_col = slot0 // 128
                        s_tile = spool.tile([128, W], mdt,
                                            tag="s", name=f"s_{b}_{c}_{t}")
                        nc.vector.tensor_scalar(
                            out=s_tile[:],
                            in0=iota_sb[:],
                            scalar1=dst_sb[:, gt_col : gt_col + 1],
                            scalar2=cnt_sb[:, gt_col : gt_col + 1],
                            op0=mybir.AluOpType.is_equal,
                            op1=mybir.AluOpType.mult,
                        )
                        nc.tensor.matmul(
                            out=agg[:],
                            lhsT=gb[:, tin, :],
                            rhs=s_tile[:],
                            start=(mm == 0),
                            stop=(mm == n_mm - 1),
                        )
                        mm += 1

                if _transposed_out(mode):
                    # transposed finalize: outp [out_d, W]
                    outp = psB.tile([out_d, W], f32, space="PSUM",
                                    tag="outp", name=f"outp_{b}")
                    if n_mm > 0:
                        aggc = fpool.tile([D, W], mdt,
                                          tag="aggc", name=f"aggc_{b}")
                        nc.scalar.copy(out=aggc[:], in_=agg[:])
                        nc.tensor.matmul(out=outp[:], lhsT=wl_sb[:],
                                         rhs=aggc[:], start=True, stop=False)
                        nc.tensor.matmul(
                            out=outp[:], lhsT=wr_sb[:],
                            rhs=xT_sb[:, b * W : (b + 1) * W],
                            start=False, stop=False)
                    else:
                        nc.tensor.matmul(
                            out=outp[:], lhsT=wr_sb[:],
                            rhs=xT_sb[:, b * W : (b + 1) * W],
                            start=True, stop=False)
                    nc.tensor.matmul(out=outp[:], lhsT=b_sb[:],
                                     rhs=ones_sb[:], start=False, stop=True)

                    fin = fpool.tile([out_d, W], out_dt,
                                     tag="fin", name=f"fin_{b}")
                    if relu:
                        nc.vector.tensor_scalar(
                            out=fin[:], in0=outp[:], scalar1=0.0,
                            scalar2=None, op0=mybir.AluOpType.max)
                    else:
                        nc.vector.tensor_copy(out=fin[:], in_=outp[:])
                    nc.sync.dma_start(out=out[:, b * W : (b + 1) * W],
                                      in_=fin[:])
                else:
                    outp = psB.tile([W, out_d], f32, space="PSUM",
                                    tag="outp", name=f"outp_{b}")
                    if n_mm > 0:
                        aggc = fpool.tile([D, W], f32,
                                          tag="aggc", name=f"aggc_{b}")
                        nc.scalar.copy(out=aggc[:], in_=agg[:])
                        nc.tensor.matmul(out=outp[:], lhsT=aggc[:],
                                         rhs=wl_sb[:], start=True, stop=False)
                        nc.tensor.matmul(
                            out=outp[:],
                            lhsT=xT_sb[:, b * W : (b + 1) * W],
                            rhs=wr_sb[:], start=False, stop=False)
                    else:
                        nc.tensor.matmul(
                            out=outp[:],
                            lhsT=xT_sb[:, b * W : (b + 1) * W],
                            rhs=wr_sb[:], start=True, stop=False)
                    nc.tensor.matmul(out=outp[:], lhsT=ones_sb[:],
                                     rhs=b_sb[:], start=False, stop=True)

                    fin = fpool.tile([W, out_d], out_dt,
                                     tag="fin", name=f"fin_{b}")
                    if relu:
                        nc.vector.tensor_scalar(
                            out=fin[:], in0=outp[:], scalar1=0.0,
                            scalar2=None, op0=mybir.AluOpType.max)
                    else:
                        nc.vector.tensor_copy(out=fin[:], in_=outp[:])
                    nc.sync.dma_start(out=out[b * W : (b + 1) * W, :],
                                      in_=fin[:])
            stack.close()

    nc.compile()
    names = dict(table=table.name, idx16=idx16.name, dstloc=dstloc.name,
                 cntinv=cntinv.name, xT=xT.name, wl=wl.name, wr=wr.name,
                 brow=brow.name, iota=iota_in.name, onesr=onesr.name,
                 out=out.name)
    return nc, names


def _layer_in_maps(names, plan, table_np, xT_np, wlT, wrT, b_vec, out_d):
    W = plan["W"]
    fdt = _feat_np_dtype(plan["mode"])
    iota = np.broadcast_to(np.arange(W, dtype=fdt), (128, W)).copy()
    in_maps = []
    for c in range(plan["n_cores"]):
        in_maps.append({
            names["table"]: np.ascontiguousarray(table_np, dtype=fdt),
            names["idx16"]: plan["idx16"][c],
            names["dstloc"]: plan["dstloc"][c],
            names["cntinv"]: plan["cntinv"][c],
            names["xT"]: np.ascontiguousarray(xT_np[c], dtype=fdt),
            names["wl"]: np.ascontiguousarray(wlT, dtype=fdt),
            names["wr"]: np.ascontiguousarray(wrT, dtype=fdt),
            names["brow"]: np.ascontiguousarray(
                b_vec.reshape(1, out_d), dtype=fdt),
            names["iota"]: iota,
            names["onesr"]: np.ones((1, W), fdt),
        })
    return in_maps


def _run_layer(nc, names, plan, table_np, xT_np, wlT, wrT, b_vec, out_d):
    in_maps = _layer_in_maps(names, plan, table_np, xT_np, wlT, wrT, b_vec,
                             out_d)
    res = bass_utils.run_bass_kernel_spmd(
        nc, in_maps, core_ids=list(range(plan["n_cores"])))
    return [res.results[c][names["out"]] for c in range(plan["n_cores"])]


def _get_plan_and_progs(edge_index):
    key = (hash(edge_index.tobytes()), MODE)
    if key not in _plan_cache:
        _plan_cache[key] = _make_plan(edge_index, N_NODES, N_CORES,
                                      CHUNK_SZ, N_CHUNKS, MODE)
    plan = _plan_cache[key]
    if (key, "L1") not in _prog_cache:
        _prog_cache[(key, "L1")] = _build_program(plan, N_NODES, HID_DIM,
                                                  relu=True)
    if (key, "L2") not in _prog_cache:
        _prog_cache[(key, "L2")] = _build_program(plan, N_NODES, OUT_DIM,
                                                  relu=False)
    return plan, _prog_cache[(key, "L1")], _prog_cache[(key, "L2")]


def kernel(x, edge_index, W1l, b1, W1r, W2l, b2, W2r):
    x = np.asarray(x, np.float32)
    edge_index = np.asarray(edge_index)
    W1l = np.asarray(W1l, np.float32)
    b1 = np.asarray(b1, np.float32)
    W1r = np.asarray(W1r, np.float32)
    W2l = np.asarray(W2l, np.float32)
    b2 = np.asarray(b2, np.float32)
    W2r = np.asarray(W2r, np.float32)

    plan, (nc1, names1), (nc2, names2) = _get_plan_and_progs(edge_index)

    slot_of_node = plan["slot_of_node"]
    spc = plan["slots_per_core"]
    n_cores = plan["n_cores"]
    fdt = _feat_np_dtype(plan["mode"])

    xq = np.zeros((n_cores * spc, IN_DIM), fdt)
    xq[slot_of_node] = x.astype(fdt)
    xT_np = [np.ascontiguousarray(xq[c * spc : (c + 1) * spc].T)
             for c in range(n_cores)]

    h_parts = _run_layer(nc1, names1, plan, x, xT_np,
                         np.ascontiguousarray(W1l.T),
                         np.ascontiguousarray(W1r.T), b1, HID_DIM)

    if _transposed_out(plan["mode"]):
        # parts are transposed [hid, spc]
        hq = np.concatenate(h_parts, axis=1)          # [hid, n_cores*spc]
        h_full = np.ascontiguousarray(hq.T[slot_of_node])
        hT_np = [np.ascontiguousarray(p) for p in h_parts]
    else:
        hq = np.concatenate(h_parts, axis=0)
        h_full = np.ascontiguousarray(hq[slot_of_node])
        hT_np = [np.ascontiguousarray(h_parts[c].T) for c in range(n_cores)]

    out_parts = _run_layer(nc2, names2, plan, h_full, hT_np,
                           np.ascontiguousarray(W2l.T),
                           np.ascontiguousarray(W2r.T), b2, OUT_DIM)
    if _transposed_out(plan["mode"]):
        oq = np.concatenate(out_parts, axis=1)        # [out_d, total]
        return np.ascontiguousarray(oq.T[slot_of_node]).astype(np.float32)
    oq = np.concatenate(out_parts, axis=0)
    return np.ascontiguousarray(oq[slot_of_node]).astype(np.float32)


# revision 6
# speedup vs baseline: 1.0893x; 1.0893x over previous
"""2-layer GraphSAGE (mean aggr) on 8 Trainium2 NeuronCores.

Strategy: partition destination nodes across cores (graph parallel).
Segment-mean is computed as TensorE matmuls: for each tile of 128 gathered
source rows M [128e, D], a routing matrix S [128e, W] (one-hot by local
destination, scaled by 1/deg) accumulates aggT[k, d] += M.T @ S into PSUM
per W-node destination block. Source rows are fetched with dma_gather
(int16 indices -> 4 source chunks of 25000 rows). Linear layers and bias
are applied per block on TensorE. Layer 1 and layer 2 run as two launches;
the host assembles the full hidden table in between (the inter-core
exchange).

Node->slot assignment: nodes are degree-sorted into bands of
n_cores*W; within each band a greedy 4-vector (per-source-chunk
in-degree) balancer splits nodes across the 8 cores so each
(block, chunk) cell has near-equal edge counts on every core. This
minimizes the shared gather-tile padding (T = max over cores).

MODE "f16": W=256 destination windows; features, routing matrices and
weights in fp16 (PE: 1 cycle/row; PSUM accumulates fp32; ~5e-4 rel err).
Block outputs are produced transposed [out_d, W]; layer-1 output is
written fp16 and reused directly as the layer-2 gather table.
MODE "f32r": fp32r datapath (~1e-4). MODE "f32": everything fp32.
"""

import contextlib
import sys

sys.path.insert(0, "/opt/trn_rl_repo")

import numpy as np

import concourse.mybir as mybir
import concourse.tile as tile
from concourse import bacc, bass_utils

N_NODES = 100000
N_EDGES = 1600000
IN_DIM = 128
HID_DIM = 128
OUT_DIM = 64
N_CORES = 8
N_CHUNKS = 4
CHUNK_SZ = 25000
GATHER_MAX = 1024  # HW limit: dma_gather wedges above this

MODE = "f16"       # "f32" | "f32r" | "f16"

_plan_cache: dict = {}
_prog_cache: dict = {}


def _block_w(mode):
    return 128 if mode == "f32" else 256


def _transposed_out(mode):
    return mode in ("f32r", "f16")


def _assign_slots(deg4, n_nodes, n_cores, W):
    """Greedy per-band balance of per-chunk degree vectors across cores.

    Returns (slot_of_node, n_bands). Band j holds the degree-ranked nodes
    [j*n_cores*W, ...); within the band each core gets W nodes chosen to
    equalize the per-chunk edge counts (which become the gather cells).
    """
    tot = deg4.sum(1)
    order = np.argsort(-tot, kind="stable")
    band_sz = n_cores * W
    n_bands = -(-n_nodes // band_sz)
    spc = n_bands * W
    slot_of_node = np.empty(n_nodes, np.int64)
    BIG = np.int64(1) << 60
    for j in range(n_bands):
        nodes = order[j * band_sz: (j + 1) * band_sz]
        s = np.zeros((n_cores, deg4.shape[1]), np.int64)
        cap = np.zeros(n_cores, np.int64)
        for n_ in nodes:
            v = deg4[n_]
            cand = s + v
            post = np.maximum(cand, s.max(0))
            score = post.sum(1)
            score[cap >= W] = BIG
            k = int(score.argmin())
            slot_of_node[n_] = k * spc + j * W + cap[k]
            s[k] += v
            cap[k] += 1
    return slot_of_node, n_bands


def _make_plan(edge_index, n_nodes, n_cores, chunk_sz, n_chunks, mode=MODE):
    src = np.asarray(edge_index[0], dtype=np.int64)
    dst = np.asarray(edge_index[1], dtype=np.int64)
    n_edges = src.shape[0]
    W = _block_w(mode)

    deg = np.bincount(dst, minlength=n_nodes).astype(np.int64)
    cnt_inv = (1.0 / np.maximum(deg, 1)).astype(np.float32)

    chunk_e = src // chunk_sz
    deg4 = np.bincount(dst * n_chunks + chunk_e,
                       minlength=n_nodes * n_chunks
                       ).reshape(n_nodes, n_chunks)
    slot_of_node, bpc = _assign_slots(deg4, n_nodes, n_cores, W)
    slots_per_core = bpc * W

    dslot = slot_of_node[dst]
    core_e = dslot // slots_per_core
    blk_e = (dslot % slots_per_core) // W
    dloc_e = dslot % W

    cell = (core_e * bpc + blk_e) * n_chunks + chunk_e
    n_cells = n_cores * bpc * n_chunks
    counts = np.bincount(cell, minlength=n_cells).reshape(
        n_cores, bpc, n_chunks)
    T = -(-counts.max(axis=0) // 128)            # [bpc, n_chunks] tiles/cell

    cell_slots = (T * 128).astype(np.int64)
    seg_len = cell_slots.sum(axis=0)             # per chunk
    seg_start = np.concatenate([[0], np.cumsum(seg_len)[:-1]])
    cell_base = np.empty((bpc, n_chunks), np.int64)
    for c in range(n_chunks):
        cell_base[:, c] = seg_start[c] + np.concatenate(
            [[0], np.cumsum(cell_slots[:, c])[:-1]])
    total_slots = int(cell_slots.sum())

    gathers = []
    for c in range(n_chunks):
        lst = []
        off = 0
        while off < seg_len[c]:
            n = int(min(GATHER_MAX, seg_len[c] - off))
            lst.append((int(seg_start[c] + off), n))
            off += n
        gathers.append(lst)

    # slot position of every edge
    eorder = np.argsort(cell, kind="stable")
    sorted_cell = cell[eorder]
    group_start = np.zeros(n_edges, np.int64)
    new_grp = np.empty(n_edges, bool)
    new_grp[0] = True
    new_grp[1:] = sorted_cell[1:] != sorted_cell[:-1]
    grp_first = np.where(new_grp)[0]
    group_start[grp_first] = grp_first
    group_start = np.maximum.accumulate(group_start)
    rank = np.arange(n_edges) - group_start

    b_of = (sorted_cell // n_chunks) % bpc
    c_of = sorted_cell % n_chunks
    core_of = sorted_cell // (bpc * n_chunks)
    pos = cell_base[b_of, c_of] + rank

    idx_vals = np.zeros((n_cores, total_slots), np.int16)
    dloc_vals = np.full((n_cores, total_slots), -1.0, np.float32)
    cinv_vals = np.zeros((n_cores, total_slots), np.float32)

    es, ed = src[eorder], dst[eorder]
    idx_vals[core_of, pos] = (es - c_of * chunk_sz).astype(np.int16)
    dloc_vals[core_of, pos] = dloc_e[eorder].astype(np.float32)
    cinv_vals[core_of, pos] = cnt_inv[ed]

    idx16 = np.ascontiguousarray(
        np.tile(idx_vals.reshape(n_cores, -1, 16).transpose(0, 2, 1),
                (1, 8, 1)))
    dstloc = np.ascontiguousarray(
        dloc_vals.reshape(n_cores, -1, 128).transpose(0, 2, 1))
    cntinv = np.ascontiguousarray(
        cinv_vals.reshape(n_cores, -1, 128).transpose(0, 2, 1))

    return dict(
        slot_of_node=slot_of_node, bpc=bpc, slots_per_core=slots_per_core,
        T=T, gathers=gathers, total_slots=total_slots,
        cell_base=cell_base, seg_start=seg_start, mode=mode, W=W,
        idx16=idx16, dstloc=dstloc, cntinv=cntinv, chunk_sz=chunk_sz,
        n_chunks=n_chunks, n_nodes=n_nodes, n_cores=n_cores,
    )


def _feat_np_dtype(mode):
    return np.float16 if mode == "f16" else np.float32


def _build_program(plan, table_rows, out_d, relu, loop_k=1, out_f32=None,
                   ablate=()):
    """One layer's SPMD program (shared by all cores).

    loop_k > 1 wraps the block loop in a hardware For loop repeating the
    computation loop_k times (timing only). out_f32 forces the DRAM
    output dtype (default: f32 unless mode f16 and relu, i.e. layer 1,
    whose output feeds the next layer's f16 gather table).
    ablate: timing-ablation set for microbenchmarks; any of
    {"gather", "stile", "matmul", "finalize"} drops that component
    (results become garbage; timing only).
    """
    ablate = set(ablate)
    bpc = plan["bpc"]
    T = plan["T"]
    n_chunks = plan["n_chunks"]
    chunk_sz = plan["chunk_sz"]
    total_slots = plan["total_slots"]
    slots_pc = plan["slots_per_core"]
    gathers = plan["gathers"]
    cell_base = plan["cell_base"]
    seg_start = plan["seg_start"]
    mode = plan["mode"]
    W = plan["W"]
    D = 128
    f32 = mybir.dt.float32
    if mode == "f32r":
        mdt = mybir.dt.float32r
    elif mode == "f16":
        mdt = mybir.dt.float16
    else:
        mdt = f32
    if out_f32 is None:
        out_f32 = not (mode == "f16" and relu)
    out_dt = f32 if out_f32 else mdt
    # self-term inputs: f16 mode runs them in f16 (1 cyc/row); f32r keeps f32
    sdt = mdt if mode == "f16" else f32

    nc = bacc.Bacc("TRN2", target_bir_lowering=False, debug=False)
    with tile.TileContext(nc) as tc:
        with tc.tile_pool(name="dram", bufs=1, space="DRAM") as dram:
            table = dram.tile([table_rows, D], mdt,
                              kind="ExternalInput", name="table")
            idx16 = dram.tile([128, total_slots // 16], mybir.dt.int16,
                              kind="ExternalInput", name="idx16")
            dstloc = dram.tile([128, total_slots // 128], f32,
                               kind="ExternalInput", name="dstloc")
            cntinv = dram.tile([128, total_slots // 128], f32,
                               kind="ExternalInput", name="cntinv")
            xT = dram.tile([D, slots_pc], sdt,
                           kind="ExternalInput", name="xT")
            wl = dram.tile([D, out_d], mdt,
                           kind="ExternalInput", name="wl")
            wr = dram.tile([D, out_d], sdt,
                           kind="ExternalInput", name="wr")
            brow = dram.tile([1, out_d], mdt,
                             kind="ExternalInput", name="brow")
            iota_in = dram.tile([128, W], mdt,
                                kind="ExternalInput", name="iota")
            onesr = dram.tile([1, W], mdt,
                              kind="ExternalInput", name="onesr")
            if _transposed_out(mode):
                out = dram.tile([out_d, slots_pc], out_dt,
                                kind="ExternalOutput", name="out")
            else:
                out = dram.tile([slots_pc, out_d], out_dt,
                                kind="ExternalOutput", name="out")

        with tc.tile_pool(name="const", bufs=1) as cpool, \
             tc.tile_pool(name="gbuf", bufs=4) as gpool, \
             tc.tile_pool(name="spool", bufs=4) as spool, \
             tc.tile_pool(name="fpool", bufs=3) as fpool, \
             tc.tile_pool(name="psA", bufs=2, space="PSUM") as psA, \
             tc.tile_pool(name="psB", bufs=2, space="PSUM") as psB:

            idx_sb = cpool.tile([128, total_slots // 16], mybir.dt.int16)
            dst_sb = cpool.tile([128, total_slots // 128], f32)
            cnt_sb = cpool.tile([128, total_slots // 128], f32)
            xT_sb = cpool.tile([D, slots_pc], sdt)
            wl_sb = cpool.tile([D, out_d], mdt)
            wr_sb = cpool.tile([D, out_d], sdt)
            b_sb = cpool.tile([1, out_d], mdt)
            ones_sb = cpool.tile([1, W], mdt)
            iota_sb = cpool.tile([128, W], mdt)

            nc.sync.dma_start(out=idx_sb[:], in_=idx16[:])
            nc.sync.dma_start(out=dst_sb[:], in_=dstloc[:])
            nc.sync.dma_start(out=cnt_sb[:], in_=cntinv[:])
            nc.sync.dma_start(out=xT_sb[:], in_=xT[:])
            nc.sync.dma_start(out=wl_sb[:], in_=wl[:])
            nc.sync.dma_start(out=wr_sb[:], in_=wr[:])
            nc.sync.dma_start(out=b_sb[:], in_=brow[:])
            nc.sync.dma_start(out=iota_sb[:], in_=iota_in[:])
            nc.sync.dma_start(out=ones_sb[:], in_=onesr[:])

            loop_ctx = (tc.For_i(0, loop_k, 1) if loop_k > 1
                        else contextlib.nullcontext())

            gtiles = [dict() for _ in range(n_chunks)]
            next_g = [0] * n_chunks

            def ensure_gather(c, gi):
                while next_g[c] <= gi:
                    g = next_g[c]
                    s0, n = gathers[c][g]
                    gb = gpool.tile([128, GATHER_MAX // 128, D], mdt,
                                    tag=f"g{c}", name=f"gb_{c}_{g}")
                    if "gather" not in ablate:
                        nc.gpsimd.dma_gather(
                            out_ap=gb[:, : -(-n // 128), :],
                            in_ap=table[c * chunk_sz : min((c + 1) * chunk_sz,
                                                           table_rows), :],
                            idxs_ap=idx_sb[:, s0 // 16 : (s0 + n) // 16],
                            num_idxs=n,
                            num_idxs_reg=n,
                            elem_size=D,
                        )
                    gtiles[c][g] = gb
                    next_g[c] = g + 1

            stack = contextlib.ExitStack()
            stack.enter_context(loop_ctx)
            for b in range(bpc):
                agg = psA.tile([D, W], f32, space="PSUM",
                               tag="agg", name=f"agg_{b}")
                n_mm = int(T[b].sum())
                mm = 0
                for c in range(n_chunks):
                    tcount = int(T[b, c])
                    for t in range(tcount):
                        slot0 = int(cell_base[b, c]) + t * 128
                        g = (slot0 - int(seg_start[c])) // GATHER_MAX
                        tin = ((slot0 - int(seg_start[c])) % GATHER_MAX) // 128
                        ensure_gather(c, g)
                        gb = gtiles[c][g]
                        gt_col = slot0 // 128
                        s_tile = spool.tile([128, W], mdt,
                                            tag="s", name=f"s_{b}_{c}_{t}")
                        if "stile" not in ablate:
                            nc.vector.tensor_scalar(
                                out=s_tile[:],
                                in0=iota_sb[:],
                                scalar1=dst_sb[:, gt_col : gt_col + 1],
                                scalar2=cnt_sb[:, gt_col : gt_col + 1],
                                op0=mybir.AluOpType.is_equal,
                                op1=mybir.AluOpType.mult,
                            )
                        if "matmul" not in ablate:
                            nc.tensor.matmul(
                                out=agg[:],
                                lhsT=gb[:, tin, :],
                                rhs=s_tile[:],
                                start=(mm == 0),
                                stop=(mm == n_mm - 1),
                            )
                        mm += 1

                if "finalize" in ablate:
                    fin = fpool.tile([out_d, W], out_dt,
                                     tag="fin", name=f"fin_{b}")
                    nc.vector.tensor_copy(out=fin[:], in_=agg[:out_d, :])
                    nc.sync.dma_start(out=out[:, b * W : (b + 1) * W],
                                      in_=fin[:])
                elif _transposed_out(mode):
                    # transposed finalize: outp [out_d, W]
                    outp = psB.tile([out_d, W], f32, space="PSUM",
                                    tag="outp", name=f"outp_{b}")
                    if n_mm > 0:
                        aggc = fpool.tile([D, W], mdt,
                                          tag="aggc", name=f"aggc_{b}")
                        nc.scalar.copy(out=aggc[:], in_=agg[:])
                        nc.tensor.matmul(out=outp[:], lhsT=wl_sb[:],
                                         rhs=aggc[:], start=True, stop=False)
                        nc.tensor.matmul(
                            out=outp[:], lhsT=wr_sb[:],
                            rhs=xT_sb[:, b * W : (b + 1) * W],
                            start=False, stop=False)
                    else:
                        nc.tensor.matmul(
                            out=outp[:], lhsT=wr_sb[:],
                            rhs=xT_sb[:, b * W : (b + 1) * W],
                            start=True, stop=False)
                    nc.tensor.matmul(out=outp[:], lhsT=b_sb[:],
                                     rhs=ones_sb[:], start=False, stop=True)

                    fin = fpool.tile([out_d, W], out_dt,
                                     tag="fin", name=f"fin_{b}")
                    if relu:
                        nc.vector.tensor_scalar(
                            out=fin[:], in0=outp[:], scalar1=0.0,
                            scalar2=None, op0=mybir.AluOpType.max)
                    else:
                        nc.vector.tensor_copy(out=fin[:], in_=outp[:])
                    nc.sync.dma_start(out=out[:, b * W : (b + 1) * W],
                                      in_=fin[:])
                else:
                    outp = psB.tile([W, out_d], f32, space="PSUM",
                                    tag="outp", name=f"outp_{b}")
                    if n_mm > 0:
                        aggc = fpool.tile([D, W], f32,
                                          tag="aggc", name=f"aggc_{b}")
                        nc.scalar.copy(out=aggc[:], in_=agg[:])
                        nc.tensor.matmul(out=outp[:], lhsT=aggc[:],
                                         rhs=wl_sb[:], start=True, stop=False)
                        nc.tensor.matmul(
                            out=outp[:],
                            lhsT=xT_sb[:, b * W : (b + 1) * W],
                            rhs=wr_sb[:], start=False, stop=False)
                    else:
                        nc.tensor.matmul(
                            out=outp[:],
                            lhsT=xT_sb[:, b * W : (b + 1) * W],
                            rhs=wr_sb[:], start=True, stop=False)
                    nc.tensor.matmul(out=outp[:], lhsT=ones_sb[:],
                                     rhs=b_sb[:], start=False, stop=True)

                    fin = fpool.tile([W, out_d], out_dt,
                                     tag="fin", name=f"fin_{b}")
                    if relu:
                        nc.vector.tensor_scalar(
                            out=fin[:], in0=outp[:], scalar1=0.0,
                            scalar2=None, op0=mybir.AluOpType.max)
                    else:
                        nc.vector.tensor_copy(out=fin[:], in_=outp[:])
                    nc.sync.dma_start(out=out[b * W : (b + 1) * W, :],
                                      in_=fin[:])
            stack.close()

    nc.compile()
    names = dict(table=table.name, idx16=idx16.name, dstloc=dstloc.name,
                 cntinv=cntinv.name, xT=xT.name, wl=wl.name, wr=wr.name,
                 brow=brow.name, iota=iota_in.name, onesr=onesr.name,
                 out=out.name)
    return nc, names


def _layer_in_maps(names, plan, table_np, xT_np, wlT, wrT, b_vec, out_d):
    W = plan["W"]
    fdt = _feat_np_dtype(plan["mode"])
    iota = np.broadcast_to(np.arange(W, dtype=fdt), (128, W)).copy()
    in_maps = []
    for c in range(plan["n_cores"]):
        in_maps.append({
            names["table"]: np.ascontiguousarray(table_np, dtype=fdt),
            names["idx16"]: plan["idx16"][c],
            names["dstloc"]: plan["dstloc"][c],
            names["cntinv"]: plan["cntinv"][c],
            names["xT"]: np.ascontiguousarray(xT_np[c], dtype=fdt),
            names["wl"]: np.ascontiguousarray(wlT, dtype=fdt),
            names["wr"]: np.ascontiguousarray(wrT, dtype=fdt),
            names["brow"]: np.ascontiguousarray(
                b_vec.reshape(1, out_d), dtype=fdt),
            names["iota"]: iota,
            names["onesr"]: np.ones((1, W), fdt),
        })
    return in_maps


def _run_layer(nc, names, plan, table_np, xT_np, wlT, wrT, b_vec, out_d):
    in_maps = _layer_in_maps(names, plan, table_np, xT_np, wlT, wrT, b_vec,
                             out_d)
    res = bass_utils.run_bass_kernel_spmd(
        nc, in_maps, core_ids=list(range(plan["n_cores"])))
    return [res.results[c][names["out"]] for c in range(plan["n_cores"])]


def _get_plan_and_progs(edge_index):
    key = (hash(edge_index.tobytes()), MODE)
    if key not in _plan_cache:
        _plan_cache[key] = _make_plan(edge_index, N_NODES, N_CORES,
                                      CHUNK_SZ, N_CHUNKS, MODE)
    plan = _plan_cache[key]
    if (key, "L1") not in _prog_cache:
        _prog_cache[(key, "L1")] = _build_program(plan, N_NODES, HID_DIM,
                                                  relu=True)
    if (key, "L2") not in _prog_cache:
        _prog_cache[(key, "L2")] = _build_program(plan, N_NODES, OUT_DIM,
                                                  relu=False)
    return plan, _prog_cache[(key, "L1")], _prog_cache[(key, "L2")]


def kernel(x, edge_index, W1l, b1, W1r, W2l, b2, W2r):
    x = np.asarray(x, np.float32)
    edge_index = np.asarray(edge_index)
    W1l = np.asarray(W1l, np.float32)
    b1 = np.asarray(b1, np.float32)
    W1r = np.asarray(W1r, np.float32)
    W2l = np.asarray(W2l, np.float32)
    b2 = np.asarray(b2, np.float32)
    W2r = np.asarray(W2r, np.float32)

    plan, (nc1, names1), (nc2, names2) = _get_plan_and_progs(edge_index)

    slot_of_node = plan["slot_of_node"]
    spc = plan["slots_per_core"]
    n_cores = plan["n_cores"]
    fdt = _feat_np_dtype(plan["mode"])

    xq = np.zeros((n_cores * spc, IN_DIM), fdt)
    xq[slot_of_node] = x.astype(fdt)
    xT_np = [np.ascontiguousarray(xq[c * spc : (c + 1) * spc].T)
             for c in range(n_cores)]

    h_parts = _run_layer(nc1, names1, plan, x, xT_np,
                         np.ascontiguousarray(W1l.T),
                         np.ascontiguousarray(W1r.T), b1, HID_DIM)

    if _transposed_out(plan["mode"]):
        # parts are transposed [hid, spc]
        hq = np.concatenate(h_parts, axis=1)          # [hid, n_cores*spc]
        h_full = np.ascontiguousarray(hq.T[slot_of_node])
        hT_np = [np.ascontiguousarray(p) for p in h_parts]
    else:
        hq = np.concatenate(h_parts, axis=0)
        h_full = np.ascontiguousarray(hq[slot_of_node])
        hT_np = [np.ascontiguousarray(h_parts[c].T) for c in range(n_cores)]

    out_parts = _run_layer(nc2, names2, plan, h_full, hT_np,
                           np.ascontiguousarray(W2l.T),
                           np.ascontiguousarray(W2r.T), b2, OUT_DIM)
    if _transposed_out(plan["mode"]):
        oq = np.concatenate(out_parts, axis=1)        # [out_d, total]
        return np.ascontiguousarray(oq.T[slot_of_node]).astype(np.float32)
    oq = np.concatenate(out_parts, axis=0)
    return np.ascontiguousarray(oq[slot_of_node]).astype(np.float32)


# revision 20
# speedup vs baseline: 1.5011x; 1.3780x over previous
"""2-layer GraphSAGE (mean aggr) on 8 Trainium2 NeuronCores.

Strategy: partition destination nodes across cores (graph parallel).
Segment-mean is computed as TensorE matmuls: for each tile of 128 gathered
source rows M [128e, D], a routing matrix S [128e, W] (one-hot by local
destination, scaled by 1/deg) accumulates aggT[k, d] += M.T @ S into PSUM
per W-node destination block. Source rows are fetched with dma_gather
(int16 indices -> 4 source chunks of 25000 rows). Linear layers and bias
are applied per block on TensorE. Layer 1 and layer 2 run as two launches;
the host assembles the full hidden table in between (the inter-core
exchange).

Node->slot assignment: nodes are degree-sorted into bands of
n_cores*W; within each band a greedy 4-vector (per-source-chunk
in-degree) balancer splits nodes across the 8 cores so each
(block, chunk) cell has near-equal edge counts on every core. This
minimizes the shared gather-tile padding (T = max over cores).

MODE "f16": W=256 destination windows; features, routing matrices and
weights in fp16 (PE: 1 cycle/row; PSUM accumulates fp32; ~5e-4 rel err).
Block outputs are produced transposed [out_d, W]; layer-1 output is
written fp16 and reused directly as the layer-2 gather table.
MODE "f32r": fp32r datapath (~1e-4). MODE "f32": everything fp32.
"""

import contextlib
import sys

sys.path.insert(0, "/opt/trn_rl_repo")

import numpy as np

import concourse.mybir as mybir
import concourse.tile as tile
from concourse import bacc, bass_utils

N_NODES = 100000
N_EDGES = 1600000
IN_DIM = 128
HID_DIM = 128
OUT_DIM = 64
N_CORES = 8
N_CHUNKS = 4
CHUNK_SZ = 25000
GATHER_MAX = 1024  # HW limit: dma_gather wedges above this
N_QUEUES = 4       # SWDGE queues (ucode max 4); 1 queue serializes the
                   # ~8.7us per-gather round trip, 4 queues pipeline it

MODE = "f16"       # "f32" | "f32r" | "f16"

_plan_cache: dict = {}
_prog_cache: dict = {}


def _block_w(mode):
    return 128 if mode == "f32" else 256


def _transposed_out(mode):
    return mode in ("f32r", "f16")


def _assign_slots(deg4, n_nodes, n_cores, W):
    """Greedy per-band balance of per-chunk degree vectors across cores.

    Returns (slot_of_node, n_bands). Band j holds the degree-ranked nodes
    [j*n_cores*W, ...); within the band each core gets W nodes chosen to
    equalize the per-chunk edge counts (which become the gather cells).
    """
    tot = deg4.sum(1)
    order = np.argsort(-tot, kind="stable")
    band_sz = n_cores * W
    n_bands = -(-n_nodes // band_sz)
    spc = n_bands * W
    slot_of_node = np.empty(n_nodes, np.int64)
    BIG = np.int64(1) << 60
    for j in range(n_bands):
        nodes = order[j * band_sz: (j + 1) * band_sz]
        s = np.zeros((n_cores, deg4.shape[1]), np.int64)
        cap = np.zeros(n_cores, np.int64)
        for n_ in nodes:
            v = deg4[n_]
            cand = s + v
            post = np.maximum(cand, s.max(0))
            score = post.sum(1)
            score[cap >= W] = BIG
            k = int(score.argmin())
            slot_of_node[n_] = k * spc + j * W + cap[k]
            s[k] += v
            cap[k] += 1
    return slot_of_node, n_bands


def _make_plan(edge_index, n_nodes, n_cores, chunk_sz, n_chunks, mode=MODE,
               gather_max=GATHER_MAX):
    src = np.asarray(edge_index[0], dtype=np.int64)
    dst = np.asarray(edge_index[1], dtype=np.int64)
    n_edges = src.shape[0]
    W = _block_w(mode)

    deg = np.bincount(dst, minlength=n_nodes).astype(np.int64)
    cnt_inv = (1.0 / np.maximum(deg, 1)).astype(np.float32)

    chunk_e = src // chunk_sz
    deg4 = np.bincount(dst * n_chunks + chunk_e,
                       minlength=n_nodes * n_chunks
                       ).reshape(n_nodes, n_chunks)
    slot_of_node, bpc = _assign_slots(deg4, n_nodes, n_cores, W)
    slots_per_core = bpc * W

    dslot = slot_of_node[dst]
    core_e = dslot // slots_per_core
    blk_e = (dslot % slots_per_core) // W
    dloc_e = dslot % W

    cell = (core_e * bpc + blk_e) * n_chunks + chunk_e
    n_cells = n_cores * bpc * n_chunks
    counts = np.bincount(cell, minlength=n_cells).reshape(
        n_cores, bpc, n_chunks)
    T = -(-counts.max(axis=0) // 128)            # [bpc, n_chunks] tiles/cell

    cell_slots = (T * 128).astype(np.int64)
    seg_len = cell_slots.sum(axis=0)             # per chunk
    seg_start = np.concatenate([[0], np.cumsum(seg_len)[:-1]])
    cell_base = np.empty((bpc, n_chunks), np.int64)
    for c in range(n_chunks):
        cell_base[:, c] = seg_start[c] + np.concatenate(
            [[0], np.cumsum(cell_slots[:, c])[:-1]])
    total_slots = int(cell_slots.sum())

    gathers = []
    for c in range(n_chunks):
        lst = []
        off = 0
        while off < seg_len[c]:
            n = int(min(gather_max, seg_len[c] - off))
            lst.append((int(seg_start[c] + off), n))
            off += n
        gathers.append(lst)

    # slot position of every edge
    eorder = np.argsort(cell, kind="stable")
    sorted_cell = cell[eorder]
    group_start = np.zeros(n_edges, np.int64)
    new_grp = np.empty(n_edges, bool)
    new_grp[0] = True
    new_grp[1:] = sorted_cell[1:] != sorted_cell[:-1]
    grp_first = np.where(new_grp)[0]
    group_start[grp_first] = grp_first
    group_start = np.maximum.accumulate(group_start)
    rank = np.arange(n_edges) - group_start

    b_of = (sorted_cell // n_chunks) % bpc
    c_of = sorted_cell % n_chunks
    core_of = sorted_cell // (bpc * n_chunks)
    pos = cell_base[b_of, c_of] + rank

    idx_vals = np.zeros((n_cores, total_slots), np.int16)
    dloc_vals = np.full((n_cores, total_slots), -1.0, np.float32)
    cinv_vals = np.zeros((n_cores, total_slots), np.float32)

    es, ed = src[eorder], dst[eorder]
    idx_vals[core_of, pos] = (es - c_of * chunk_sz).astype(np.int16)
    dloc_vals[core_of, pos] = dloc_e[eorder].astype(np.float32)
    cinv_vals[core_of, pos] = cnt_inv[ed]

    idx16 = np.ascontiguousarray(
        np.tile(idx_vals.reshape(n_cores, -1, 16).transpose(0, 2, 1),
                (1, 8, 1)))
    dstloc = np.ascontiguousarray(
        dloc_vals.reshape(n_cores, -1, 128).transpose(0, 2, 1))
    cntinv = np.ascontiguousarray(
        cinv_vals.reshape(n_cores, -1, 128).transpose(0, 2, 1))

    return dict(
        slot_of_node=slot_of_node, bpc=bpc, slots_per_core=slots_per_core,
        T=T, gathers=gathers, total_slots=total_slots,
        cell_base=cell_base, seg_start=seg_start, mode=mode, W=W,
        idx16=idx16, dstloc=dstloc, cntinv=cntinv, chunk_sz=chunk_sz,
        n_chunks=n_chunks, n_nodes=n_nodes, n_cores=n_cores,
        gather_max=gather_max,
    )


def _feat_np_dtype(mode):
    return np.float16 if mode == "f16" else np.float32


def _build_program(plan, table_rows, out_d, relu, loop_k=1, out_f32=None,
                   ablate=(), n_queues=N_QUEUES, gbufs=4):
    """One layer's SPMD program (shared by all cores).

    loop_k > 1 wraps the block loop in a hardware For loop repeating the
    computation loop_k times (timing only). out_f32 forces the DRAM
    output dtype (default: f32 unless mode f16 and relu, i.e. layer 1,
    whose output feeds the next layer's f16 gather table).
    ablate: timing-ablation set for microbenchmarks; any of
    {"gather", "stile", "matmul", "finalize"} drops that component
    (results become garbage; timing only).
    """
    ablate = set(ablate)
    bpc = plan["bpc"]
    T = plan["T"]
    n_chunks = plan["n_chunks"]
    chunk_sz = plan["chunk_sz"]
    total_slots = plan["total_slots"]
    slots_pc = plan["slots_per_core"]
    gathers = plan["gathers"]
    cell_base = plan["cell_base"]
    seg_start = plan["seg_start"]
    mode = plan["mode"]
    W = plan["W"]
    D = 128
    f32 = mybir.dt.float32
    if mode == "f32r":
        mdt = mybir.dt.float32r
    elif mode == "f16":
        mdt = mybir.dt.float16
    else:
        mdt = f32
    if out_f32 is None:
        out_f32 = not (mode == "f16" and relu)
    out_dt = f32 if out_f32 else mdt
    # self-term inputs: f16 mode runs them in f16 (1 cyc/row); f32r keeps f32
    sdt = mdt if mode == "f16" else f32
    gather_max = plan.get("gather_max", GATHER_MAX)

    nc = bacc.Bacc(
        "TRN2", target_bir_lowering=False, debug=False,
        dynamic_dma_scratch_size=max(16384, 16 * gather_max),
        num_swdge_queues=n_queues,
    )
    with tile.TileContext(nc) as tc:
        with tc.tile_pool(name="dram", bufs=1, space="DRAM") as dram:
            table = dram.tile([table_rows, D], mdt,
                              kind="ExternalInput", name="table")
            idx16 = dram.tile([128, total_slots // 16], mybir.dt.int16,
                              kind="ExternalInput", name="idx16")
            dstloc = dram.tile([128, total_slots // 128], f32,
                               kind="ExternalInput", name="dstloc")
            cntinv = dram.tile([128, total_slots // 128], f32,
                               kind="ExternalInput", name="cntinv")
            xT = dram.tile([D, slots_pc], sdt,
                           kind="ExternalInput", name="xT")
            wl = dram.tile([D, out_d], mdt,
                           kind="ExternalInput", name="wl")
            wr = dram.tile([D, out_d], sdt,
                           kind="ExternalInput", name="wr")
            brow = dram.tile([1, out_d], mdt,
                             kind="ExternalInput", name="brow")
            iota_in = dram.tile([128, W], mdt,
                                kind="ExternalInput", name="iota")
            onesr = dram.tile([1, W], mdt,
                              kind="ExternalInput", name="onesr")
            if _transposed_out(mode):
                out = dram.tile([out_d, slots_pc], out_dt,
                                kind="ExternalOutput", name="out")
            else:
                out = dram.tile([slots_pc, out_d], out_dt,
                                kind="ExternalOutput", name="out")

        with tc.tile_pool(name="const", bufs=1) as cpool, \
             tc.tile_pool(name="gbuf", bufs=gbufs) as gpool, \
             tc.tile_pool(name="spool", bufs=4) as spool, \
             tc.tile_pool(name="fpool", bufs=3) as fpool, \
             tc.tile_pool(name="psA", bufs=2, space="PSUM") as psA, \
             tc.tile_pool(name="psB", bufs=2, space="PSUM") as psB:

            idx_sb = cpool.tile([128, total_slots // 16], mybir.dt.int16)
            dst_sb = cpool.tile([128, total_slots // 128], f32)
            cnt_sb = cpool.tile([128, total_slots // 128], f32)
            xT_sb = cpool.tile([D, slots_pc], sdt)
            wl_sb = cpool.tile([D, out_d], mdt)
            wr_sb = cpool.tile([D, out_d], sdt)
            b_sb = cpool.tile([1, out_d], mdt)
            ones_sb = cpool.tile([1, W], mdt)
            iota_sb = cpool.tile([128, W], mdt)

            nc.sync.dma_start(out=idx_sb[:], in_=idx16[:])
            nc.sync.dma_start(out=dst_sb[:], in_=dstloc[:])
            nc.sync.dma_start(out=cnt_sb[:], in_=cntinv[:])
            nc.sync.dma_start(out=xT_sb[:], in_=xT[:])
            nc.sync.dma_start(out=wl_sb[:], in_=wl[:])
            nc.sync.dma_start(out=wr_sb[:], in_=wr[:])
            nc.sync.dma_start(out=b_sb[:], in_=brow[:])
            nc.sync.dma_start(out=iota_sb[:], in_=iota_in[:])
            nc.sync.dma_start(out=ones_sb[:], in_=onesr[:])

            loop_ctx = (tc.For_i(0, loop_k, 1) if loop_k > 1
                        else contextlib.nullcontext())

            gtiles = [dict() for _ in range(n_chunks)]
            next_g = [0] * n_chunks

            gcounter = [0]

            def ensure_gather(c, gi):
                while next_g[c] <= gi:
                    g = next_g[c]
                    s0, n = gathers[c][g]
                    gb = gpool.tile([128, gather_max // 128, D], mdt,
                                    tag=f"g{c}", name=f"gb_{c}_{g}")
                    if "gather" not in ablate:
                        nc.gpsimd.dma_gather(
                            out_ap=gb[:, : -(-n // 128), :],
                            in_ap=table[c * chunk_sz : min((c + 1) * chunk_sz,
                                                           table_rows), :],
                            idxs_ap=idx_sb[:, s0 // 16 : (s0 + n) // 16],
                            num_idxs=n,
                            num_idxs_reg=n,
                            elem_size=D,
                            queue_num=gcounter[0] % n_queues,
                        )
                        gcounter[0] += 1
                    gtiles[c][g] = gb
                    next_g[c] = g + 1

            stack = contextlib.ExitStack()
            stack.enter_context(loop_ctx)
            for b in range(bpc):
                if "matmul" not in ablate:
                    agg = psA.tile([D, W], f32, space="PSUM",
                                   tag="agg", name=f"agg_{b}")
                n_mm = int(T[b].sum())
                mm = 0
                for c in range(n_chunks):
                    tcount = int(T[b, c])
                    for t in range(tcount):
                        slot0 = int(cell_base[b, c]) + t * 128
                        g = (slot0 - int(seg_start[c])) // gather_max
                        tin = ((slot0 - int(seg_start[c])) % gather_max) // 128
                        ensure_gather(c, g)
                        gb = gtiles[c][g]
                        gt_col = slot0 // 128
                        s_tile = spool.tile([128, W], mdt,
                                            tag="s", name=f"s_{b}_{c}_{t}")
                        if "stile" not in ablate:
                            nc.vector.tensor_scalar(
                                out=s_tile[:],
                                in0=iota_sb[:],
                                scalar1=dst_sb[:, gt_col : gt_col + 1],
                                scalar2=cnt_sb[:, gt_col : gt_col + 1],
                                op0=mybir.AluOpType.is_equal,
                                op1=mybir.AluOpType.mult,
                            )
                        if "matmul" not in ablate:
                            nc.tensor.matmul(
                                out=agg[:],
                                lhsT=gb[:, tin, :],
                                rhs=s_tile[:],
                                start=(mm == 0),
                                stop=(mm == n_mm - 1),
                            )
                        mm += 1

                if "finalize" in ablate:
                    fin = fpool.tile([out_d, W], out_dt,
                                     tag="fin", name=f"fin_{b}")
                    nc.scalar.copy(out=fin[:], in_=iota_sb[:out_d, :])
                    nc.sync.dma_start(out=out[:, b * W : (b + 1) * W],
                                      in_=fin[:])
                elif _transposed_out(mode):
                    # transposed finalize: outp [out_d, W]
                    outp = psB.tile([out_d, W], f32, space="PSUM",
                                    tag="outp", name=f"outp_{b}")
                    if n_mm > 0:
                        aggc = fpool.tile([D, W], mdt,
                                          tag="aggc", name=f"aggc_{b}")
                        nc.scalar.copy(out=aggc[:], in_=agg[:])
                        nc.tensor.matmul(out=outp[:], lhsT=wl_sb[:],
                                         rhs=aggc[:], start=True, stop=False)
                        nc.tensor.matmul(
                            out=outp[:], lhsT=wr_sb[:],
                            rhs=xT_sb[:, b * W : (b + 1) * W],
                            start=False, stop=False)
                    else:
                        nc.tensor.matmul(
                            out=outp[:], lhsT=wr_sb[:],
                            rhs=xT_sb[:, b * W : (b + 1) * W],
                            start=True, stop=False)
                    nc.tensor.matmul(out=outp[:], lhsT=b_sb[:],
                                     rhs=ones_sb[:], start=False, stop=True)

                    fin = fpool.tile([out_d, W], out_dt,
                                     tag="fin", name=f"fin_{b}")
                    if relu:
                        nc.vector.tensor_scalar(
                            out=fin[:], in0=outp[:], scalar1=0.0,
                            scalar2=None, op0=mybir.AluOpType.max)
                    else:
                        nc.vector.tensor_copy(out=fin[:], in_=outp[:])
                    nc.sync.dma_start(out=out[:, b * W : (b + 1) * W],
                                      in_=fin[:])
                else:
                    outp = psB.tile([W, out_d], f32, space="PSUM",
                                    tag="outp", name=f"outp_{b}")
                    if n_mm > 0:
                        aggc = fpool.tile([D, W], f32,
                                          tag="aggc", name=f"aggc_{b}")
                        nc.scalar.copy(out=aggc[:], in_=agg[:])
                        nc.tensor.matmul(out=outp[:], lhsT=aggc[:],
                                         rhs=wl_sb[:], start=True, stop=False)
                        nc.tensor.matmul(
                            out=outp[:],
                            lhsT=xT_sb[:, b * W : (b + 1) * W],
                            rhs=wr_sb[:], start=False, stop=False)
                    else:
                        nc.tensor.matmul(
                            out=outp[:],
                            lhsT=xT_sb[:, b * W : (b + 1) * W],
                            rhs=wr_sb[:], start=True, stop=False)
                    nc.tensor.matmul(out=outp[:], lhsT=ones_sb[:],
                                     rhs=b_sb[:], start=False, stop=True)

                    fin = fpool.tile([W, out_d], out_dt,
                                     tag="fin", name=f"fin_{b}")
                    if relu:
                        nc.vector.tensor_scalar(
                            out=fin[:], in0=outp[:], scalar1=0.0,
                            scalar2=None, op0=mybir.AluOpType.max)
                    else:
                        nc.vector.tensor_copy(out=fin[:], in_=outp[:])
                    nc.sync.dma_start(out=out[b * W : (b + 1) * W, :],
                                      in_=fin[:])
            stack.close()

    nc.compile()
    names = dict(table=table.name, idx16=idx16.name, dstloc=dstloc.name,
                 cntinv=cntinv.name, xT=xT.name, wl=wl.name, wr=wr.name,
                 brow=brow.name, iota=iota_in.name, onesr=onesr.name,
                 out=out.name)
    return nc, names


def _layer_in_maps(names, plan, table_np, xT_np, wlT, wrT, b_vec, out_d):
    W = plan["W"]
    fdt = _feat_np_dtype(plan["mode"])
    iota = np.broadcast_to(np.arange(W, dtype=fdt), (128, W)).copy()
    in_maps = []
    for c in range(plan["n_cores"]):
        in_maps.append({
            names["table"]: np.ascontiguousarray(table_np, dtype=fdt),
            names["idx16"]: plan["idx16"][c],
            names["dstloc"]: plan["dstloc"][c],
            names["cntinv"]: plan["cntinv"][c],
            names["xT"]: np.ascontiguousarray(xT_np[c], dtype=fdt),
            names["wl"]: np.ascontiguousarray(wlT, dtype=fdt),
            names["wr"]: np.ascontiguousarray(wrT, dtype=fdt),
            names["brow"]: np.ascontiguousarray(
                b_vec.reshape(1, out_d), dtype=fdt),
            names["iota"]: iota,
            names["onesr"]: np.ones((1, W), fdt),
        })
    return in_maps


def _run_layer(nc, names, plan, table_np, xT_np, wlT, wrT, b_vec, out_d):
    in_maps = _layer_in_maps(names, plan, table_np, xT_np, wlT, wrT, b_vec,
                             out_d)
    res = bass_utils.run_bass_kernel_spmd(
        nc, in_maps, core_ids=list(range(plan["n_cores"])))
    return [res.results[c][names["out"]] for c in range(plan["n_cores"])]


def _get_plan_and_progs(edge_index):
    key = (hash(edge_index.tobytes()), MODE)
    if key not in _plan_cache:
        _plan_cache[key] = _make_plan(edge_index, N_NODES, N_CORES,
                                      CHUNK_SZ, N_CHUNKS, MODE)
    plan = _plan_cache[key]
    if (key, "L1") not in _prog_cache:
        _prog_cache[(key, "L1")] = _build_program(plan, N_NODES, HID_DIM,
                                                  relu=True)
    if (key, "L2") not in _prog_cache:
        _prog_cache[(key, "L2")] = _build_program(plan, N_NODES, OUT_DIM,
                                                  relu=False)
    return plan, _prog_cache[(key, "L1")], _prog_cache[(key, "L2")]


def kernel(x, edge_index, W1l, b1, W1r, W2l, b2, W2r):
    x = np.asarray(x, np.float32)
    edge_index = np.asarray(edge_index)
    W1l = np.asarray(W1l, np.float32)
    b1 = np.asarray(b1, np.float32)
    W1r = np.asarray(W1r, np.float32)
    W2l = np.asarray(W2l, np.float32)
    b2 = np.asarray(b2, np.float32)
    W2r = np.asarray(W2r, np.float32)

    plan, (nc1, names1), (nc2, names2) = _get_plan_and_progs(edge_index)

    slot_of_node = plan["slot_of_node"]
    spc = plan["slots_per_core"]
    n_cores = plan["n_cores"]
    fdt = _feat_np_dtype(plan["mode"])

    xq = np.zeros((n_cores * spc, IN_DIM), fdt)
    xq[slot_of_node] = x.astype(fdt)
    xT_np = [np.ascontiguousarray(xq[c * spc : (c + 1) * spc].T)
             for c in range(n_cores)]

    h_parts = _run_layer(nc1, names1, plan, x, xT_np,
                         np.ascontiguousarray(W1l.T),
                         np.ascontiguousarray(W1r.T), b1, HID_DIM)

    if _transposed_out(plan["mode"]):
        # parts are transposed [hid, spc]
        hq = np.concatenate(h_parts, axis=1)          # [hid, n_cores*spc]
        h_full = np.ascontiguousarray(hq.T[slot_of_node])
        hT_np = [np.ascontiguousarray(p) for p in h_parts]
    else:
        hq = np.concatenate(h_parts, axis=0)
        h_full = np.ascontiguousarray(hq[slot_of_node])
        hT_np = [np.ascontiguousarray(h_parts[c].T) for c in range(n_cores)]

    out_parts = _run_layer(nc2, names2, plan, h_full, hT_np,
                           np.ascontiguousarray(W2l.T),
                           np.ascontiguousarray(W2r.T), b2, OUT_DIM)
    if _transposed_out(plan["mode"]):
        oq = np.concatenate(out_parts, axis=1)        # [out_d, total]
        return np.ascontiguousarray(oq.T[slot_of_node]).astype(np.float32)
    oq = np.concatenate(out_parts, axis=0)
    return np.ascontiguousarray(oq[slot_of_node]).astype(np.float32)
